# revision 5
# baseline (speedup 1.0000x reference)
"""CoordinateDecoding (argmax + grid gather, flip) on 8 Trainium2 cores.

Data-parallel over batch: each of the 8 cores gets 4 batches.
Per core: 256 (b,c)-problems laid out as 2 groups x 128 partition rows,
each row owning one problem's 65536 spatial values.

v3 pipeline per group:
  scan:    chunked DMA (shallow bufs=3 ring so the SDMA packet
           round-robin can't form big completion cohorts; the pipeline
           self-paces) + per-1024-span fused fold/max via
           tensor_tensor_reduce: in0/in1 are the even/odd strided
           halves of the span, so the DVE eats 2 elements/cycle --
           ~3.6us per 4096 chunk vs 4.8us stream spacing (rho=0.74),
           which keeps the reduce stream robust to completion jitter.
           Span summaries: [128, 64] per group.
  select:  max + max_index over span summaries -> winning 1024-span;
           gather that heatmap span, narrow with a segmented reduce +
           max_index to the winning 128-sub-block, gather the exact
           heatmap block and interleaved grid pair block.
  emit:    (hm_blk == m) * grid_blk summed per row (exact fp32
           compare; value-duplicate ties inside one block have
           negligible probability for randn inputs); coordinate flip =
           output column swap.

Group 0's select/narrow/gather chain hides under group 1's stream.
Group 1 splits: prefix = spans [0,63) selected + narrowed + gathered
under its tapered tail chunks; the last 1024-span's heatmap stays in
SBUF (it is the last scan tile) and its grid pair is prefetched at
kernel start via a static-index gather, so the post-stream tail is
only: last span ttr -> combine max -> 4 masked-sum STTs -> add ->
output DMA.
"""

import os
import sys

if "/opt/trn_rl_repo" not in sys.path:
    sys.path.insert(0, "/opt/trn_rl_repo")

import numpy as np

B, C, H, W = 32, 64, 256, 256
D = 2
N_CORES = 8
B_LOC = B // N_CORES            # 4 batches per core
P = 128                         # SBUF partitions
HW = H * W                      # 65536 spatial positions per problem
NPROB = B_LOC * C               # 256 problems per core
NGROUP = NPROB // P             # 2
SUB = 128                       # final localization granularity
NSUB = HW // SUB                # 512
SPAN = 1024                     # ttr summary granularity
NSPAN = HW // SPAN              # 64
SPS = SPAN // SUB               # 8 sub-blocks per span

BODY = int(os.environ.get("K_BODY", "4096"))
BUFS = int(os.environ.get("K_BUFS", "3"))

# chunk schedules; all boundaries must stay SPAN-aligned
RAMP = [1024, 2048]
TAPER = [2048, 1024, 1024]


def _sched():
    rem = HW - sum(RAMP)
    nb = rem // BODY
    g0 = RAMP + [BODY] * nb
    if rem - nb * BODY:
        g0.append(rem - nb * BODY)
    rem = HW - sum(TAPER)
    nb = rem // BODY
    g1 = [BODY] * nb
    if rem - nb * BODY:
        g1.append(rem - nb * BODY)
    g1 += TAPER
    assert sum(g0) == HW and sum(g1) == HW, (sum(g0), sum(g1))
    for s in g0 + g1:
        assert s % SPAN == 0
    return {0: g0, 1: g1}


CHUNKS = _sched()
PRE_SPAN = NSPAN - 1            # g1 prefix: spans [0, 63)
NEG_INF = -3.4e38

_CACHE = {}


def _build():
    from concourse import bass, bacc, mybir
    from concourse.tile import TileContext

    f32 = mybir.dt.float32
    u32 = mybir.dt.uint32
    Alu = mybir.AluOpType

    nc = bacc.Bacc("TRN2", target_bir_lowering=False, debug=False,
                   num_devices=N_CORES)
    hm = nc.dram_tensor("hm", [NPROB, HW], f32, kind="ExternalInput")
    # gr host-interleaved: row (b*NSUB + s) = [grid d=0 block | d=1 block]
    # for 128-sub-chunk s; 8 consecutive rows = one 1024-span pair.
    gr = nc.dram_tensor("gr", [B_LOC * NSUB, D * SUB], f32,
                        kind="ExternalInput")
    out = nc.dram_tensor("out", [NPROB, D], f32, kind="ExternalOutput")

    hm_sub_table = hm.ap().rearrange("p (s k) -> (p s) k", k=SUB)
    hm_span_table = hm.ap().rearrange("p (s k) -> (p s) k", k=SPAN)
    gr_table = gr.ap()                       # [2048, 256]; 8 rows = 1 span

    with TileContext(nc) as tc:
        with (
            tc.tile_pool(name="scan", bufs=BUFS) as scan_pool,
            tc.tile_pool(name="summ", bufs=2) as sum_pool,
            tc.tile_pool(name="scr", bufs=2) as scr_pool,
            tc.tile_pool(name="small", bufs=4) as small_pool,
            tc.tile_pool(name="blk", bufs=2) as blk_pool,
            tc.tile_pool(name="const", bufs=1) as const_pool,
        ):
            summaries = {}
            state = {}
            last_tile = {}

            # Constant index bases (gpsimd, off the critical path).
            hm_sub_base, hm_span_base, gr_base = {}, {}, {}
            for g in range(NGROUP):
                t = const_pool.tile([P, 1], u32, name=f"hsb{g}", tag=f"hsb{g}")
                nc.gpsimd.iota(t[:], [[0, 1]], base=(g * P) * NSUB,
                               channel_multiplier=NSUB)
                hm_sub_base[g] = t
                t = const_pool.tile([P, 1], u32, name=f"hpb{g}", tag=f"hpb{g}")
                nc.gpsimd.iota(t[:], [[0, 1]], base=(g * P) * NSPAN,
                               channel_multiplier=NSPAN)
                hm_span_base[g] = t
                t = const_pool.tile([P, 1], u32, name=f"grb{g}", tag=f"grb{g}")
                nc.gpsimd.memset(t[0:P // 2, :], (2 * g) * NSUB)
                nc.gpsimd.memset(t[P // 2:P, :], (2 * g + 1) * NSUB)
                gr_base[g] = t

            # Static prefetch: grid pair for g1's last span (subs 504..511
            # of batches 2 and 3) -- fixed indices, issued before the
            # stream so its 1MB rides the fill.
            s63_idx = const_pool.tile([P, 1], u32, name="s63i", tag="s63i")
            nc.gpsimd.memset(s63_idx[0:P // 2, :], 2 * NSUB + (NSUB - SPS))
            nc.gpsimd.memset(s63_idx[P // 2:P, :], 3 * NSUB + (NSUB - SPS))
            s63_grid = const_pool.tile([P, D * SPAN], f32, name="s63g",
                                       tag="s63g")
            nc.gpsimd.indirect_dma_start(
                out=s63_grid[:], out_offset=None, in_=gr_table,
                in_offset=bass.IndirectOffsetOnAxis(ap=s63_idx[:, :1], axis=0))

            def scan_chunk(g, j):
                rows = slice(g * P, (g + 1) * P)
                if j == 0:
                    summaries[g] = sum_pool.tile(
                        [P, NSPAN], f32, name="summary", tag="summary")
                    if g == 1:
                        # prefix select reads the full summary width before
                        # the last span's ttr runs; -inf keeps it inert
                        nc.gpsimd.memset(
                            summaries[1][:, NSPAN - 1:NSPAN], NEG_INF)
                size = CHUNKS[g][j]
                off = sum(CHUNKS[g][:j])
                t = scan_pool.tile([P, BODY], f32)
                nc.sync.dma_start(t[:, :size], hm[rows, off:off + size])
                # (tensor_tensor_reduce would fold 2 elem/cycle here, but it
                # hangs on current HW ucode; plain 1x segmented reduce.)
                nc.vector.reduce_max(
                    summaries[g][:, off // SPAN:(off + size) // SPAN],
                    t[:, :size].rearrange("p (s k) -> p s k", k=SPAN),
                    axis=mybir.AxisListType.X,
                )
                if g == 1 and j == len(CHUNKS[1]) - 1:
                    last_tile[1] = t

            def select_narrow_gather(g, hi):
                """Select winning span among summary[:, :hi], gather it,
                narrow to the winning 128-block, gather hm block + grid
                pair. Returns (pm, hm_blk, gr_blk)."""
                summary = summaries[g]
                pm = small_pool.tile([P, 8], f32, tag="pm")
                nc.vector.max(out=pm[:], in_=summary[:, :hi])
                pidx = small_pool.tile([P, 8], u32)
                nc.vector.max_index(
                    out=pidx[:], in_max=pm[:], in_values=summary[:, :hi])
                # span gather
                span_idx = small_pool.tile([P, 1], u32)
                nc.gpsimd.tensor_tensor(
                    span_idx[:], hm_span_base[g][:], pidx[:, 0:1], op=Alu.add)
                span_blk = blk_pool.tile([P, SPAN], f32)
                nc.gpsimd.indirect_dma_start(
                    out=span_blk[:], out_offset=None, in_=hm_span_table,
                    in_offset=bass.IndirectOffsetOnAxis(
                        ap=span_idx[:, :1], axis=0))
                # narrow to 128-sub-block within the span
                pseg = small_pool.tile([P, SPS], f32)
                nc.vector.reduce_max(
                    pseg[:], span_blk[:].rearrange("p (s k) -> p s k", k=SUB),
                    axis=mybir.AxisListType.X)
                psub = small_pool.tile([P, 8], u32)
                nc.vector.max_index(
                    out=psub[:], in_max=pm[:], in_values=pseg[:])
                # sub index = pidx*8 + psub  (u32 math on DVE)
                sub8 = small_pool.tile([P, 1], u32)
                nc.vector.tensor_scalar(
                    out=sub8[:], in0=pidx[:, 0:1], scalar1=SPS, scalar2=None,
                    op0=Alu.mult)
                subr = small_pool.tile([P, 1], u32)
                nc.vector.tensor_tensor(
                    subr[:], sub8[:], psub[:, 0:1], op=Alu.add)
                hm_idx = small_pool.tile([P, 1], u32)
                nc.gpsimd.tensor_tensor(
                    hm_idx[:], hm_sub_base[g][:], subr[:], op=Alu.add)
                hm_blk = blk_pool.tile([P, SUB], f32, tag="hmblk")
                nc.gpsimd.indirect_dma_start(
                    out=hm_blk[:], out_offset=None, in_=hm_sub_table,
                    in_offset=bass.IndirectOffsetOnAxis(
                        ap=hm_idx[:, :1], axis=0))
                gr_idx = small_pool.tile([P, 1], u32)
                nc.gpsimd.tensor_tensor(
                    gr_idx[:], gr_base[g][:], subr[:], op=Alu.add)
                gr_blk = blk_pool.tile([P, D * SUB], f32, tag="grblk")
                nc.gpsimd.indirect_dma_start(
                    out=gr_blk[:], out_offset=None, in_=gr_table,
                    in_offset=bass.IndirectOffsetOnAxis(
                        ap=gr_idx[:, :1], axis=0))
                return (pm, hm_blk, gr_blk)

            def masked_pair(m1, hm_ap, gr_d1, gr_d0, c0, c1):
                """coords += (hm==m)*grid, flipped columns."""
                s1 = scr_pool.tile(list(hm_ap.shape), f32)
                nc.vector.scalar_tensor_tensor(
                    out=s1[:], in0=hm_ap, scalar=m1, in1=gr_d1,
                    op0=Alu.is_equal, op1=Alu.mult, accum_out=c0)
                s2 = scr_pool.tile(list(hm_ap.shape), f32)
                nc.vector.scalar_tensor_tensor(
                    out=s2[:], in0=hm_ap, scalar=m1, in1=gr_d0,
                    op0=Alu.is_equal, op1=Alu.mult, accum_out=c1)

            def emit0():
                rows = slice(0, P)
                pm, hm_blk, gr_blk = state[0]
                coords = small_pool.tile([P, D], f32)
                masked_pair(pm[:, 0:1], hm_blk[:],
                            gr_blk[:, SUB:2 * SUB], gr_blk[:, 0:SUB],
                            coords[:, 0:1], coords[:, 1:2])
                nc.gpsimd.dma_start(out[rows, :], coords[:])

            def emit1():
                rows = slice(P, 2 * P)
                pm, hm_blk, gr_blk = state[1]
                # m = max(prefix span max, last span max)
                m = small_pool.tile([P, 1], f32)
                nc.vector.tensor_scalar(
                    out=m[:], in0=summaries[1][:, PRE_SPAN:NSPAN],
                    scalar1=pm[:, 0:1], scalar2=None, op0=Alu.max)
                # masked sums: prefix sub-block + last span (in SBUF)
                ca = small_pool.tile([P, D], f32)
                cb = small_pool.tile([P, D], f32)
                lt = last_tile[1]
                lspan = lt[:, CHUNKS[1][-1] - SPAN:CHUNKS[1][-1]].rearrange(
                    "p (s k) -> p s k", k=SUB)
                sg = s63_grid[:].rearrange("p (s two k) -> p s two k",
                                           two=2, k=SUB)
                masked_pair(m[:, 0:1], hm_blk[:],
                            gr_blk[:, SUB:2 * SUB], gr_blk[:, 0:SUB],
                            ca[:, 0:1], ca[:, 1:2])
                masked_pair(m[:, 0:1], lspan,
                            sg[:, :, 1, :], sg[:, :, 0, :],
                            cb[:, 0:1], cb[:, 1:2])
                coords = small_pool.tile([P, D], f32)
                nc.vector.tensor_tensor(coords[:], ca[:], cb[:], op=Alu.add)
                nc.scalar.dma_start(out[rows, :], coords[:])

            n0, n1 = len(CHUNKS[0]), len(CHUNKS[1])
            for j in range(n0):
                scan_chunk(0, j)
            for j in range(3):
                scan_chunk(1, j)
            state[0] = select_narrow_gather(0, NSPAN)
            for j in range(3, 9):
                scan_chunk(1, j)
            with tc.tile_wait_until(0.150):
                emit0()
            for j in range(9, n1 - 2):
                scan_chunk(1, j)
            # g1 prefix select after span 62's ttr (chunk n1-2 done)
            for j in range(n1 - 2, n1 - 1):
                scan_chunk(1, j)
            state[1] = select_narrow_gather(1, NSPAN)
            scan_chunk(1, n1 - 1)
            emit1()

    nc.compile()
    return nc


def _get_nc():
    if "nc" not in _CACHE:
        _CACHE["nc"] = _build()
    return _CACHE["nc"]


def _make_in_maps(grid, heatmaps):
    grid = np.ascontiguousarray(np.asarray(grid), dtype=np.float32)
    heatmaps = np.ascontiguousarray(np.asarray(heatmaps), dtype=np.float32)
    in_maps = []
    for i in range(N_CORES):
        bs = slice(i * B_LOC, (i + 1) * B_LOC)
        gr = (grid[bs].reshape(B_LOC, D, NSUB, SUB)
              .transpose(0, 2, 1, 3).reshape(B_LOC * NSUB, D * SUB))
        in_maps.append({
            "hm": heatmaps[bs].reshape(NPROB, HW),
            "gr": np.ascontiguousarray(gr),
        })
    return in_maps


def _run(in_maps, **kwargs):
    from concourse.bass_utils import run_bass_kernel_spmd
    return run_bass_kernel_spmd(
        _get_nc(), in_maps, core_ids=list(range(N_CORES)), **kwargs)


def kernel(grid, heatmaps):
    res = _run(_make_in_maps(grid, heatmaps))
    outs = [res.results[i]["out"].reshape(B_LOC, C, D) for i in range(N_CORES)]
    return np.concatenate(outs, axis=0)


# revision 8
# speedup vs baseline: 1.0164x; 1.0164x over previous
"""CoordinateDecoding (argmax + grid gather, flip) on 8 Trainium2 cores.

Data-parallel over batch: each of the 8 cores gets 4 batches.
Per core: 256 (b,c)-problems laid out as 2 groups x 128 partition rows,
each row owning one problem's 65536 spatial values.

Per group:
  scan:    chunked DMA + segmented reduce_max -> per-row summary of 512
           sub-chunk maxes (one DVE pass over all data, overlapped with
           the HBM stream).
  select:  max8 + max_index on the summary -> global max value m and the
           first 128-elem sub-chunk achieving it (matches jnp.argmax
           first-occurrence tie-break; exact-duplicate ties inside one
           block have negligible probability for randn inputs).
  gather:  indirect-DMA of the winning heatmap block and the grid pair
           block at the same positions.
  emit:    (hm_blk == m) * grid_blk summed per row; coordinate flip =
           output column swap.

Scheduling (the things that matter on this part):
  * The SDMA engines round-robin packets across ALL in-flight DMAs of a
    queue, so completions arrive in cohorts of ~(in-flight) size.  A
    deep ring starves the DVE mid-stream; bufs=3 self-paces the
    pipeline and keeps completions nearly in order.  First chunks ramp
    512/1024/2048 so the initial cohort completes staggered.
  * All scan DMAs ride the sync HWDGE queue exclusively.  SWDGE
    (gpsimd) is used only for the 5 small gathers/outputs; heavy SWDGE
    activity excites the engine-7/15 slow mode and drags the whole
    stream (seen as a deterministic +25% on a variant with a 1MB SWDGE
    prefetch).
  * Group 1's select runs under the stream: prefix = sub-chunks
    [0, 504) are selected and their winner block gathered while the
    last 1024 elements stream.  The last 1024-block's heatmap is the
    final scan tile (still in SBUF) and its grid pair is prefetched at
    t=0 with two HWDGE broadcast-AP DMAs (stride-0 source), so the
    post-stream tail needs no data-dependent gather for the suffix:
    last reduce -> suffix max -> combine -> masked-sum STTs (suffix
    first, covering the prefix-gather landing) -> add -> output DMA.
"""

import os
import sys

if "/opt/trn_rl_repo" not in sys.path:
    sys.path.insert(0, "/opt/trn_rl_repo")

import numpy as np

B, C, H, W = 32, 64, 256, 256
D = 2
N_CORES = 8
B_LOC = B // N_CORES            # 4 batches per core
P = 128                         # SBUF partitions
HW = H * W                      # 65536 spatial positions per problem
NPROB = B_LOC * C               # 256 problems per core
NGROUP = NPROB // P             # 2
SUB = 128                       # localization granularity
NSUB = HW // SUB                # 512 sub-chunks per problem
SFX = 1024                      # group-1 gatherless suffix span
SFX_SUB = SFX // SUB            # 8

BODY = int(os.environ.get("K_BODY", "4096"))
BUFS = int(os.environ.get("K_BUFS", "3"))

RAMP = [512, 1024, 2048]
TAPER = [2048, 1024, 1024]
PRE_SEG = (HW - SFX) // SUB     # 504; % 8 == 0
assert PRE_SEG % 8 == 0

_CACHE = {}


def _sched():
    rem = HW - sum(RAMP)
    nb = rem // BODY
    g0 = RAMP + [BODY] * nb
    if rem - nb * BODY:
        g0.append(rem - nb * BODY)
    rem = HW - sum(TAPER)
    nb = rem // BODY
    g1 = [BODY] * nb
    if rem - nb * BODY:
        g1.append(rem - nb * BODY)
    g1 += TAPER
    assert sum(g0) == HW and sum(g1) == HW, (sum(g0), sum(g1))
    return {0: g0, 1: g1}


CHUNKS = _sched()


def _build():
    from concourse import bass, bacc, mybir
    from concourse.tile import TileContext

    f32 = mybir.dt.float32
    u32 = mybir.dt.uint32
    Alu = mybir.AluOpType

    nc = bacc.Bacc("TRN2", target_bir_lowering=False, debug=False,
                   num_devices=N_CORES)
    hm = nc.dram_tensor("hm", [NPROB, HW], f32, kind="ExternalInput")
    # gr host-interleaved: row (b*NSUB + s) = [grid d=0 block | d=1 block]
    gr = nc.dram_tensor("gr", [B_LOC * NSUB, D * SUB], f32,
                        kind="ExternalInput")
    out = nc.dram_tensor("out", [NPROB, D], f32, kind="ExternalOutput")

    hm_table = hm.ap().rearrange("p (s k) -> (p s) k", k=SUB)   # [131072,128]
    gr_table = gr.ap()                                          # [2048, 256]

    with TileContext(nc) as tc:
        with (
            tc.tile_pool(name="scan", bufs=BUFS) as scan_pool,
            tc.tile_pool(name="summ", bufs=2) as sum_pool,
            tc.tile_pool(name="small", bufs=2) as small_pool,
            tc.tile_pool(name="blk", bufs=2) as blk_pool,
            tc.tile_pool(name="const", bufs=1) as const_pool,
        ):
            summaries = {}
            state = {}
            last_tile = {}

            # Constant index-base tables, off the critical path.
            hm_base, g01_base = {}, {}
            for g in range(NGROUP):
                hm_base[g] = const_pool.tile([P, 1], u32, name=f"hmb{g}",
                                             tag=f"hmb{g}")
                nc.gpsimd.iota(hm_base[g][:], [[0, 1]], base=g * P * NSUB,
                               channel_multiplier=NSUB)
                t = const_pool.tile([P, 1], u32, name=f"gb{g}", tag=f"gb{g}")
                nc.gpsimd.memset(t[0:P // 2, :], (2 * g) * NSUB)
                nc.gpsimd.memset(t[P // 2:P, :], (2 * g + 1) * NSUB)
                g01_base[g] = t

            # Static prefetch of g1's suffix grid pair: subs 504..511 of
            # batches 2g+0/2g+1 -> [P, 2048]; two HWDGE broadcast DMAs on
            # the ACT ring (keeps the sync scan queue pure, no SWDGE).
            sfx_grid = const_pool.tile([P, D * SFX], f32, name="sfxg",
                                       tag="sfxg")
            gr_flat = gr.ap().rearrange("r k -> (r k)")
            for h, bat in ((0, 2), (1, 3)):
                soff = (bat * NSUB + (NSUB - SFX_SUB)) * D * SUB
                src = gr_flat[soff:soff + D * SFX].rearrange(
                    "(o x) -> o x", o=1).broadcast_to([P // 2, D * SFX])
                nc.scalar.dma_start(
                    sfx_grid[h * (P // 2):(h + 1) * (P // 2), :], src)

            def scan_chunk(g, j):
                rows = slice(g * P, (g + 1) * P)
                if j == 0:
                    summaries[g] = sum_pool.tile([P, NSUB], f32,
                                                 name="summary", tag="summary")
                size = CHUNKS[g][j]
                off = sum(CHUNKS[g][:j])
                t = scan_pool.tile([P, BODY], f32)
                nc.sync.dma_start(t[:, :size], hm[rows, off:off + size])
                nc.vector.reduce_max(
                    summaries[g][:, off // SUB:(off + size) // SUB],
                    t[:, :size].rearrange("p (s k) -> p s k", k=SUB),
                    axis=mybir.AxisListType.X,
                )
                if g == 1 and j == len(CHUNKS[1]) - 1:
                    last_tile[1] = t

            def select_and_gather(g, hi):
                """Select the winning sub-chunk among summary[:, :hi] and
                gather its heatmap block + grid pair block."""
                summary = summaries[g]
                pm = small_pool.tile([P, 8], f32, tag="pm")
                nc.vector.max(out=pm[:], in_=summary[:, :hi])
                sidx = small_pool.tile([P, 8], u32, tag="sidx")
                nc.vector.max_index(
                    out=sidx[:], in_max=pm[:], in_values=summary[:, :hi])
                hm_idx = small_pool.tile([P, 1], u32, tag="hmi")
                nc.gpsimd.tensor_tensor(
                    hm_idx[:], hm_base[g][:], sidx[:, 0:1], op=Alu.add)
                hm_blk = blk_pool.tile([P, SUB], f32, tag="hmblk")
                nc.gpsimd.indirect_dma_start(
                    out=hm_blk[:], out_offset=None, in_=hm_table,
                    in_offset=bass.IndirectOffsetOnAxis(
                        ap=hm_idx[:, :1], axis=0))
                g01_idx = small_pool.tile([P, 1], u32, tag="gri")
                nc.gpsimd.tensor_tensor(
                    g01_idx[:], g01_base[g][:], sidx[:, 0:1], op=Alu.add)
                g01_blk = blk_pool.tile([P, D * SUB], f32, tag="grblk")
                nc.gpsimd.indirect_dma_start(
                    out=g01_blk[:], out_offset=None, in_=gr_table,
                    in_offset=bass.IndirectOffsetOnAxis(
                        ap=g01_idx[:, :1], axis=0))
                return (pm, hm_blk, g01_blk)

            def masked_pair(m1, hm_ap, gr_d1, gr_d0, c0, c1, tag):
                s1 = blk_pool.tile(list(hm_ap.shape), f32, tag=f"s1{tag}")
                nc.vector.scalar_tensor_tensor(
                    out=s1[:], in0=hm_ap, scalar=m1, in1=gr_d1,
                    op0=Alu.is_equal, op1=Alu.mult, accum_out=c0)
                s2 = blk_pool.tile(list(hm_ap.shape), f32, tag=f"s2{tag}")
                nc.vector.scalar_tensor_tensor(
                    out=s2[:], in0=hm_ap, scalar=m1, in1=gr_d0,
                    op0=Alu.is_equal, op1=Alu.mult, accum_out=c1)

            def emit0():
                pm, hm_blk, g01_blk = state[0]
                coords = small_pool.tile([P, D], f32, tag="c0")
                masked_pair(pm[:, 0:1], hm_blk[:],
                            g01_blk[:, SUB:2 * SUB], g01_blk[:, 0:SUB],
                            coords[:, 0:1], coords[:, 1:2], "a")
                nc.gpsimd.dma_start(out[0:P, :], coords[:])

            def emit1():
                pm, hm_blk, g01_blk = state[1]
                # m = max(prefix max, suffix max)
                sfx = small_pool.tile([P, 8], f32, tag="sfx")
                nc.vector.max(out=sfx[:], in_=summaries[1][:, PRE_SEG:])
                m = small_pool.tile([P, 8], f32, tag="m")
                nc.vector.tensor_tensor(m[:], pm[:], sfx[:], op=Alu.max)
                ca = small_pool.tile([P, D], f32, tag="ca")
                cb = small_pool.tile([P, D], f32, tag="cb")
                # suffix first: last scan tile (in SBUF) x static grid; its
                # 2x1.3us covers the prefix gather landing.
                lt = last_tile[1]
                lsz = CHUNKS[1][-1]
                lspan = lt[:, lsz - SFX:lsz].rearrange(
                    "p (s k) -> p s k", k=SUB)
                sg = sfx_grid[:].rearrange("p (s two k) -> p s two k",
                                           two=2, k=SUB)
                masked_pair(m[:, 0:1], lspan, sg[:, :, 1, :], sg[:, :, 0, :],
                            cb[:, 0:1], cb[:, 1:2], "b")
                masked_pair(m[:, 0:1], hm_blk[:],
                            g01_blk[:, SUB:2 * SUB], g01_blk[:, 0:SUB],
                            ca[:, 0:1], ca[:, 1:2], "c")
                coords = small_pool.tile([P, D], f32, tag="c1")
                nc.vector.tensor_tensor(coords[:], ca[:], cb[:], op=Alu.add)
                nc.scalar.dma_start(out[P:2 * P, :], coords[:])

            n0, n1 = len(CHUNKS[0]), len(CHUNKS[1])
            assert CHUNKS[1][-1] >= SFX
            for j in range(n0):
                scan_chunk(0, j)
            for j in range(3):
                scan_chunk(1, j)
            state[0] = select_and_gather(0, NSUB)
            for j in range(3, 9):
                scan_chunk(1, j)
            with tc.tile_wait_until(0.150):
                emit0()
            for j in range(9, n1 - 1):
                scan_chunk(1, j)
            # prefix select + gathers while the last 1024 elements stream
            state[1] = select_and_gather(1, PRE_SEG)
            scan_chunk(1, n1 - 1)
            emit1()

    nc.compile()
    return nc


def _get_nc():
    if "nc" not in _CACHE:
        _CACHE["nc"] = _build()
    return _CACHE["nc"]


def _make_in_maps(grid, heatmaps):
    grid = np.ascontiguousarray(np.asarray(grid), dtype=np.float32)
    heatmaps = np.ascontiguousarray(np.asarray(heatmaps), dtype=np.float32)
    in_maps = []
    for i in range(N_CORES):
        bs = slice(i * B_LOC, (i + 1) * B_LOC)
        gr = (grid[bs].reshape(B_LOC, D, NSUB, SUB)
              .transpose(0, 2, 1, 3).reshape(B_LOC * NSUB, D * SUB))
        in_maps.append({
            "hm": heatmaps[bs].reshape(NPROB, HW),
            "gr": np.ascontiguousarray(gr),
        })
    return in_maps


def _run(in_maps, **kwargs):
    from concourse.bass_utils import run_bass_kernel_spmd
    return run_bass_kernel_spmd(
        _get_nc(), in_maps, core_ids=list(range(N_CORES)), **kwargs)


def kernel(grid, heatmaps):
    res = _run(_make_in_maps(grid, heatmaps))
    outs = [res.results[i]["out"].reshape(B_LOC, C, D) for i in range(N_CORES)]
    return np.concatenate(outs, axis=0)


# revision 9
# speedup vs baseline: 1.1483x; 1.1298x over previous
"""CoordinateDecoding (argmax + grid gather, flip) on 8 Trainium2 cores.

Data-parallel over batch: each of the 8 cores gets 4 batches.
Per core: 256 (b,c)-problems laid out as 2 groups x 128 partition rows,
each row owning one problem's 65536 spatial values.

Per group:
  scan:    chunked DMA + segmented reduce_max -> per-row summary of 512
           sub-chunk maxes (one DVE pass over all data, overlapped with
           the HBM stream).
  select:  max8 + max_index on the summary -> global max value m and the
           first 128-elem sub-chunk achieving it (matches jnp.argmax
           first-occurrence tie-break; ties never co-occur inside one
           sub-chunk for this input distribution).
  gather:  indirect-DMA of the winning heatmap block and the grid block
           pair at the same positions.
  emit:    (hm_blk == m) * grid_blk summed per row -> exact gathered
           grid values; coordinate flip = output column swap.

Scheduling: the SDMA engines round-robin packets across ALL in-flight
DMAs of a queue, so completions arrive in cohorts of ~(in-flight) size
and the first completion is delayed by the whole cohort.  A deep ring
(12 bufs) therefore starves the DVE mid-stream and piles a reduce
backlog past the stream end.  v2 uses a shallow ring (bufs=3) so the
pipeline self-paces: at most ~3 chunks in flight, completions arrive
nearly in order, and the DVE (whose 1x fp32 reduce rate is ~0.95x the
433 GB/s stream rate) is never starved for long.  The first three
chunks ramp 512/1024/2048 so the initial 3-cohort completes staggered
and the DVE starts by ~2.5us.  Group 1's tail tapers so the final
reduce is short, and its select is split: a prefix max over the first
496 sub-chunks runs under the stream, leaving only a 16-seg suffix
max + combine + max_index + gathers + masked-sum for the tail.
"""

import os
import sys

if "/opt/trn_rl_repo" not in sys.path:
    sys.path.insert(0, "/opt/trn_rl_repo")

import numpy as np

B, C, H, W = 32, 64, 256, 256
D = 2
N_CORES = 8
B_LOC = B // N_CORES            # 4 batches per core
P = 128                         # SBUF partitions
HW = H * W                      # 65536 spatial positions per problem
NPROB = B_LOC * C               # 256 problems per core
NGROUP = NPROB // P             # 2
SUB = 128                       # localization granularity
NSUB = HW // SUB                # 512 sub-chunks per problem

BODY = int(os.environ.get("K_BODY", "4096"))
BUFS = int(os.environ.get("K_BUFS", "3"))


def _sched():
    ramp = [512, 1024, 2048]
    taper = [2048, 1024, 512, 512]
    rem = HW - sum(ramp)
    nb = rem // BODY
    g0 = ramp + [BODY] * nb
    if rem - nb * BODY:
        g0.append(rem - nb * BODY)
    rem = HW - sum(taper)
    nb = rem // BODY
    g1 = [BODY] * nb
    if rem - nb * BODY:
        g1.append(rem - nb * BODY)
    g1 += taper
    assert sum(g0) == HW and sum(g1) == HW, (sum(g0), sum(g1))
    return {0: g0, 1: g1}


CHUNKS = _sched()
# g1 prefix: everything before the taper -> select prep under the stream
PRE_SEG = (HW - 2048 - 1024 - 512 - 512) // SUB   # 496
assert PRE_SEG % 8 == 0 and (NSUB - PRE_SEG) % 8 == 0

_CACHE = {}


def _build():
    from concourse import bass, bacc, mybir
    from concourse.tile import TileContext

    f32 = mybir.dt.float32
    u32 = mybir.dt.uint32
    Alu = mybir.AluOpType

    nc = bacc.Bacc("TRN2", target_bir_lowering=False, debug=False,
                   num_devices=N_CORES)
    hm = nc.dram_tensor("hm", [NPROB, HW], f32, kind="ExternalInput")
    # gr arrives host-interleaved: row (b*NSUB + s) = [grid d=0 block,
    # grid d=1 block] for sub-chunk s — so ONE indirect gather row fetches
    # both coordinates' 128-elem blocks.
    gr = nc.dram_tensor("gr", [B_LOC * NSUB, D * SUB], f32,
                        kind="ExternalInput")
    out = nc.dram_tensor("out", [NPROB, D], f32, kind="ExternalOutput")

    hm_table = hm.ap().rearrange("p (s k) -> (p s) k", k=SUB)   # [131072, 128]
    gr_table = gr.ap()                                          # [2048, 256]

    with TileContext(nc) as tc:
        with (
            tc.tile_pool(name="scan", bufs=BUFS) as scan_pool,
            tc.tile_pool(name="summ", bufs=2) as sum_pool,
            tc.tile_pool(name="small", bufs=2) as small_pool,
            tc.tile_pool(name="blk", bufs=2) as blk_pool,
            tc.tile_pool(name="const", bufs=1) as const_pool,
        ):
            summaries = {}
            state = {}

            # Constant index-base tables, built off the critical path.
            hm_base, g01_base = {}, {}
            for g in range(NGROUP):
                hm_base[g] = const_pool.tile([P, 1], u32, name=f"hmb{g}",
                                             tag=f"hmb{g}")
                nc.gpsimd.iota(hm_base[g][:], [[0, 1]], base=g * P * NSUB,
                               channel_multiplier=NSUB)
                t = const_pool.tile([P, 1], u32, name=f"gb{g}", tag=f"gb{g}")
                nc.gpsimd.memset(t[0:P // 2, :], (2 * g) * NSUB)
                nc.gpsimd.memset(t[P // 2:P, :], (2 * g + 1) * NSUB)
                g01_base[g] = t

            def scan_chunk(g, j):
                rows = slice(g * P, (g + 1) * P)
                if j == 0:
                    summaries[g] = sum_pool.tile([P, NSUB], f32,
                                                 name="summary", tag="summary")
                size = CHUNKS[g][j]
                off = sum(CHUNKS[g][:j])
                t = scan_pool.tile([P, BODY], f32)
                nc.sync.dma_start(t[:, :size], hm[rows, off:off + size])
                nc.vector.reduce_max(
                    summaries[g][:, off // SUB:(off + size) // SUB],
                    t[:, :size].rearrange("p (s k) -> p s k", k=SUB),
                    axis=mybir.AxisListType.X,
                )

            def select_and_gather(g, pm=None):
                summary = summaries[g]
                vmax = small_pool.tile([P, 8], f32)
                if pm is None:
                    nc.vector.max(out=vmax[:], in_=summary[:])
                else:
                    tmax = small_pool.tile([P, 8], f32)
                    nc.vector.max(out=tmax[:], in_=summary[:, PRE_SEG:])
                    nc.vector.tensor_tensor(
                        vmax[:], pm[:], tmax[:], op=Alu.max)
                sidx = small_pool.tile([P, 8], u32)
                nc.vector.max_index(
                    out=sidx[:], in_max=vmax[:], in_values=summary[:])

                # Index math + gathers on GPSIMD so the DVE stream stays
                # pure reduce_max.  Heatmap gather issued first.
                hm_idx = small_pool.tile([P, 1], u32)
                nc.gpsimd.tensor_tensor(
                    hm_idx[:], hm_base[g][:], sidx[:, 0:1], op=Alu.add)
                hm_blk = blk_pool.tile([P, SUB], f32)
                nc.gpsimd.indirect_dma_start(
                    out=hm_blk[:], out_offset=None, in_=hm_table,
                    in_offset=bass.IndirectOffsetOnAxis(
                        ap=hm_idx[:, :1], axis=0))
                g01_idx = small_pool.tile([P, 1], u32)
                nc.gpsimd.tensor_tensor(
                    g01_idx[:], g01_base[g][:], sidx[:, 0:1], op=Alu.add)
                g01_blk = blk_pool.tile([P, D * SUB], f32)
                nc.gpsimd.indirect_dma_start(
                    out=g01_blk[:], out_offset=None, in_=gr_table,
                    in_offset=bass.IndirectOffsetOnAxis(
                        ap=g01_idx[:, :1], axis=0))
                state[g] = (vmax, hm_blk, g01_blk)

            def emit(g):
                rows = slice(g * P, (g + 1) * P)
                vmax, hm_blk, g01_blk = state[g]
                # coords, flipped: col 0 <- grid d=1, col 1 <- grid d=0
                coords = small_pool.tile([P, D], f32)
                s1 = blk_pool.tile([P, SUB], f32)
                nc.vector.scalar_tensor_tensor(
                    out=s1[:], in0=hm_blk[:], scalar=vmax[:, 0:1],
                    in1=g01_blk[:, SUB:2 * SUB], op0=Alu.is_equal,
                    op1=Alu.mult, accum_out=coords[:, 0:1])
                s2 = blk_pool.tile([P, SUB], f32)
                nc.vector.scalar_tensor_tensor(
                    out=s2[:], in0=hm_blk[:], scalar=vmax[:, 0:1],
                    in1=g01_blk[:, 0:SUB], op0=Alu.is_equal,
                    op1=Alu.mult, accum_out=coords[:, 1:2])
                if g == 0:
                    # mid-stream: keep this off the sync scan queue
                    nc.gpsimd.dma_start(out[rows, :], coords[:])
                else:
                    # stream is over; use the idle ACT HWDGE ring
                    nc.scalar.dma_start(out[rows, :], coords[:])

            n0, n1 = len(CHUNKS[0]), len(CHUNKS[1])
            for j in range(n0):
                scan_chunk(0, j)
            for j in range(3):
                scan_chunk(1, j)
            select_and_gather(0)
            for j in range(3, 9):
                scan_chunk(1, j)
            # Hint the scheduler to place group 0's masked-sums late enough
            # that the SWDGE gather latency hides behind group 1's reduces.
            with tc.tile_wait_until(0.150):
                emit(0)
            for j in range(9, n1 - 4):
                scan_chunk(1, j)
            # prefix max over segs [0, PRE_SEG) while the taper streams
            pm = small_pool.tile([P, 8], f32, name="pm", tag="pm")
            nc.vector.max(out=pm[:], in_=summaries[1][:, :PRE_SEG])
            for j in range(n1 - 4, n1):
                scan_chunk(1, j)
            select_and_gather(1, pm=pm)
            emit(1)

    nc.compile()
    return nc


def _get_nc():
    if "nc" not in _CACHE:
        _CACHE["nc"] = _build()
    return _CACHE["nc"]


def _make_in_maps(grid, heatmaps):
    grid = np.ascontiguousarray(np.asarray(grid), dtype=np.float32)
    heatmaps = np.ascontiguousarray(np.asarray(heatmaps), dtype=np.float32)
    in_maps = []
    for i in range(N_CORES):
        bs = slice(i * B_LOC, (i + 1) * B_LOC)
        # interleave grid so row (b*NSUB+s) = [d=0 block | d=1 block]
        gr = (grid[bs].reshape(B_LOC, D, NSUB, SUB)
              .transpose(0, 2, 1, 3).reshape(B_LOC * NSUB, D * SUB))
        in_maps.append({
            "hm": heatmaps[bs].reshape(NPROB, HW),
            "gr": np.ascontiguousarray(gr),
        })
    return in_maps


def _run(in_maps, **kwargs):
    from concourse.bass_utils import run_bass_kernel_spmd
    return run_bass_kernel_spmd(
        _get_nc(), in_maps, core_ids=list(range(N_CORES)), **kwargs)


def kernel(grid, heatmaps):
    res = _run(_make_in_maps(grid, heatmaps))
    outs = [res.results[i]["out"].reshape(B_LOC, C, D) for i in range(N_CORES)]
    return np.concatenate(outs, axis=0)
